# revision 24
# baseline (speedup 1.0000x reference)
"""Trainium2 Bass kernel for the Mamba-style K=2 selective-scan block.

Strategy: data-parallel over batch B=8 across the 8 NeuronCores. Per core,
everything runs in [channel, time] layout:
  - in_proj / 1x1 conv / SiLU on TensorE+ScalarE (fp32 matmuls)
  - selective scan: per (direction k, d-tile, state n): decay a = exp(A*delta)
    on ScalarE (per-partition scale), w = dtu*B_bc and the affine recurrence
    (tensor_tensor_scan) + y accumulation on VectorE, all fp16
  - time is chunked (TC=1024) with a 32-step warmup window; the per-step decay
    is >= ~0.6 nats so 32 steps of warmup reproduce the exact fp32 state,
    making chunks independent (no cross-chunk scan state)
  - B/C rows are broadcast across partitions via DRAM round-trip DMA
  - merge fwd/rev + LayerNorm + z-gate + out_proj on PE/ACT/DVE
"""
import numpy as np

B, S, DIM = 8, 4096, 256
DI, K, N, R = 512, 2, 16, 16
L = S
P = 128
DT = DI // P          # 4 d-tiles of 128 channels
TC = 1024             # output tokens per chunk
WU = 32               # warmup steps (decay >= 0.6 nats/step -> exp(-19) left)
NCH = L // TC
MM = 512              # matmul free-dim chunk

_CACHE = {}


# ----------------------------------------------------------------- patches --
def _install_patches():
    """Walrus in this container rejects >1 semaphore wait on some instruction
    structs; hoist surplus waits onto InstNoOp carriers. Also shim
    antenv.axon_hooks so trace=True can NTFF-profile (optional)."""
    if _CACHE.get("patched"):
        return
    import concourse.tile as tile
    from concourse import mybir

    MAXW = 1
    orig_commit = tile.TileContext._commit_instruction

    def split_waits(self, inst):
        si = inst.sync_info
        if si is None or not si.on_wait or len(si.on_wait) <= MAXW:
            return
        waits = list(si.on_wait)
        surplus, keep = waits[:-MAXW], waits[-MAXW:]
        eng = inst.engine
        if eng == mybir.EngineType.Unassigned:
            return
        for i in range(0, len(surplus), MAXW):
            nop = mybir.InstNoOp(name=f"{inst.name}-w{i}", ins=[], outs=[])
            nop.engine = eng
            nop.sync_info = mybir.SyncInfo(
                on_wait=surplus[i : i + MAXW], on_update=[]
            )
            self._add_instruction(nop)
        inst.sync_info = mybir.SyncInfo(on_wait=keep, on_update=list(si.on_update))

    def patched_commit(self, inst, lazy_reg_writes=True):
        split_waits(self, inst)
        return orig_commit(self, inst, lazy_reg_writes)

    def patched_drain_and_barrier(self, tick_clock, wait_clock):
        from concourse.tile import ScopedClock

        drain_inst = self.nc.sync.drain()
        wait_clock.add_sem_waits(
            drain_inst.ins, ScopedClock({None: tick_clock.global_clock})
        )
        si = drain_inst.ins.sync_info
        waits = list(si.on_wait) if si is not None and si.on_wait else []
        if len(waits) > MAXW:
            drain_inst.ins.sync_info = mybir.SyncInfo(
                on_wait=waits[:MAXW], on_update=list(si.on_update)
            )
            rest = waits[MAXW:]
            for i in range(0, len(rest), MAXW):
                extra = self.nc.sync.drain()
                extra.ins.sync_info = mybir.SyncInfo(
                    on_wait=rest[i : i + MAXW], on_update=[]
                )
        self.nc.all_engine_barrier()
        assert self.sems is not None
        popped = self.nc._tile_sem_poison_stack.pop()
        assert popped is self._sem_poison
        self.nc.clear_and_free_semaphores(list(self.sems.allocated().values()))
        self.nc.all_engine_barrier()

    tile.TileContext._commit_instruction = patched_commit
    tile.TileContext._drain_and_barrier = patched_drain_and_barrier
    _CACHE["patched"] = True


def _install_profile_shim():
    if _CACHE.get("shim"):
        return True
    try:
        import sys, types
        import antenv

        name = "antenv.axon_hooks"
        if name not in sys.modules:
            mod = types.ModuleType(name)
            hook = [None]
            mod.set_axon_ntff_profile_hook = lambda h: hook.__setitem__(0, h)
            mod.get_axon_ntff_profile_hook = lambda: hook[0]
            sys.modules[name] = mod
            antenv.axon_hooks = mod
            from trn_agent_boot.trn_boot import _ntff_profile_via_ctypes

            h = _ntff_profile_via_ctypes("/opt/axon/libaxon_pjrt.so")
            if h is not None:
                mod.set_axon_ntff_profile_hook(h)
        _CACHE["shim"] = True
        return True
    except Exception:
        return False


# ------------------------------------------------------------------- build --
def _build():
    if "nc" in _CACHE:
        return _CACHE["nc"]
    _install_patches()
    import concourse.bass as bass
    import concourse.tile as tile
    from concourse import mybir

    F32 = mybir.dt.float32
    F16 = mybir.dt.float16
    AF = mybir.ActivationFunctionType
    OP = mybir.AluOpType

    nc = bass.Bass("TRN2", target_bir_lowering=False, debug=False, num_devices=1)

    def din(name, shape, dt=F32):
        return nc.dram_tensor(name, shape, dt, kind="ExternalInput").ap()

    xT = din("xT", [DIM, L], F16)
    winT = din("winT", [DIM, 2 * DI], F16)
    convT = din("convT", [DI, DI], F16)
    convb = din("convb", [P, DT])
    nconvbd = din("nconvb", [P, DT])
    xprojT = din("xprojT", [K, DI, 96], F16)
    dtwT = din("dtwT", [K, R, DI], F16)
    dtb = din("dtb", [K, P, DT])
    Adr = din("A", [K, P, 64])
    Dsdr = din("Ds", [K, P, DT])
    lnw = din("lnw", [P, DT])
    lnb = din("lnb", [P, DT])
    woutT = din("woutT", [DI, DIM], F16)
    ones = din("ones", [P, 1], F16)
    onesr = din("onesr", [1, P], F16)
    l8d = din("l8", [8, P], F16)
    l32d = din("l32", [P, P], F16)
    ldg = din("ldiag", [K, DT, P, P], F16)
    epsd = din("eps", [P, 1])
    out_dr = nc.dram_tensor("out", [DIM, L], F32, kind="ExternalOutput").ap()

    with tile.TileContext(nc) as tc:
        cpool = tc.alloc_tile_pool(name="const", bufs=1)
        upool = tc.alloc_tile_pool(name="ufull", bufs=1)

        # ---- constants to SBUF ----
        winT_sb = []
        for kt in range(2):
            t = cpool.tile([P, 2 * DI], F16, tag=f"winT{kt}", name=f"winT{kt}")
            nc.sync.dma_start(t[:], winT[kt * P : (kt + 1) * P, :])
            winT_sb.append(t)
        convT_sb = []
        for kt in range(DT):
            t = cpool.tile([P, DI], F16, tag=f"convT{kt}", name=f"convT{kt}")
            nc.sync.dma_start(t[:], convT[kt * P : (kt + 1) * P, :])
            convT_sb.append(t)
        xprojT_sb = {}
        for k in range(K):
            for kt in range(DT):
                t = cpool.tile([P, 96], F16, tag=f"xprojT{k}{kt}", name=f"xprojT{k}{kt}")
                nc.sync.dma_start(t[:], xprojT[k, kt * P : (kt + 1) * P, :])
                xprojT_sb[k, kt] = t
        dtwT_sb = {}
        for k in range(K):
            t = cpool.tile([R, DI], F16, tag=f"dtwT{k}", name=f"dtwT{k}")
            nc.sync.dma_start(t[:], dtwT[k])
            dtwT_sb[k] = t
        woutT_sb = []
        for kt in range(DT):
            t = cpool.tile([P, DIM], F16, tag=f"woutT{kt}", name=f"woutT{kt}")
            nc.sync.dma_start(t[:], woutT[kt * P : (kt + 1) * P, :])
            woutT_sb.append(t)
        A_sb, dtb_sb, Ds_sb = {}, {}, {}
        for k in range(K):
            t = cpool.tile([P, 64], F32, tag=f"A{k}", name=f"A{k}")
            nc.sync.dma_start(t[:], Adr[k])
            A_sb[k] = t
            t = cpool.tile([P, DT], F32, tag=f"dtb{k}", name=f"dtb{k}")
            nc.sync.dma_start(t[:], dtb[k])
            dtb_sb[k] = t
            t = cpool.tile([P, DT], F32, tag=f"Ds{k}", name=f"Ds{k}")
            nc.sync.dma_start(t[:], Dsdr[k])
            Ds_sb[k] = t
        convb_sb = cpool.tile([P, DT], F32, tag="convb", name="convb")
        nc.sync.dma_start(convb_sb[:], convb[:])
        nconvb_sb = cpool.tile([P, DT], F32, tag="nconvb", name="nconvb")
        nc.sync.dma_start(nconvb_sb[:], nconvbd[:])
        lnw_sb = cpool.tile([P, DT], F32, tag="lnw", name="lnw")
        nc.sync.dma_start(lnw_sb[:], lnw[:])
        lnb_sb = cpool.tile([P, DT], F32, tag="lnb", name="lnb")
        nc.sync.dma_start(lnb_sb[:], lnb[:])
        ones_sb = cpool.tile([P, 1], F16, tag="ones", name="ones")
        nc.sync.dma_start(ones_sb[:], ones[:])
        onesr_sb = cpool.tile([1, P], F16, tag="onesr", name="onesr")
        nc.sync.dma_start(onesr_sb[:], onesr[:])
        eps_sb = cpool.tile([P, 1], F32, tag="eps", name="eps")
        nc.sync.dma_start(eps_sb[:], epsd[:])
        l8_sb = cpool.tile([8, P], F16, tag="l8", name="l8")
        nc.sync.dma_start(l8_sb[:], l8d[:])
        l32_sb = cpool.tile([P, P], F16, tag="l32", name="l32")
        nc.sync.dma_start(l32_sb[:], l32d[:])
        ldiag_sb = {}
        for k in range(K):
            for sp in range(DT):
                t = cpool.tile([P, P], F16, tag=f"ldg{k}{sp}", name=f"ldg{k}{sp}")
                nc.sync.dma_start(t[:], ldg[k, sp])
                ldiag_sb[k, sp] = t

        # persistent activations; z spills to DRAM (read back at merge);
        # u is split into per-512-column tiles for fine-grained deps
        NJ = L // MM
        u_full = [[upool.tile([P, MM], F16, tag=f"u{m}_{j}", name=f"u{m}_{j}")
                   for j in range(NJ)] for m in range(DT)]
        zdrp = tc.alloc_tile_pool(name="zdr", bufs=1, space="DRAM")
        z_dr = zdrp.tile([DI, L], F16, tag="zdr", name="zdr")

        # ---- stage 1: in_proj -> silu(z); conv1x1 -> silu -> u (per j) ----
        s1 = tc.alloc_tile_pool(name="s1", bufs=2)
        ps1 = tc.alloc_tile_pool(name="ps1", bufs=2, space="PSUM")

        def stage1_j(j):
            sl = slice(j * MM, (j + 1) * MM)
            xt = []
            for kt in range(2):
                # SWDGE queue: x loads bypass the constant-load serialization
                # on the sync queue at startup
                t = s1.tile([P, MM], F16, tag=f"xt{kt}", name=f"xt{kt}")
                nc.gpsimd.dma_start(t[:], xT[kt * P : (kt + 1) * P, sl])
                xt.append(t)
            xx_sb = []
            for m in range(DT):
                pz = ps1.tile([P, MM], F32, tag="ps1", name="ps1")
                for kt in range(2):
                    nc.tensor.matmul(
                        pz[:], winT_sb[kt][:, m * P : (m + 1) * P], xt[kt][:],
                        start=(kt == 0), stop=(kt == 1))
                t = s1.tile([P, MM], F16, tag=f"xx{m}", name=f"xx{m}")
                nc.scalar.copy(t[:], pz[:])
                xx_sb.append(t)
            for m in range(DT):
                pz = ps1.tile([P, MM], F32, tag="ps1", name="ps1")
                for kt in range(2):
                    nc.tensor.matmul(
                        pz[:], winT_sb[kt][:, DI + m * P : DI + (m + 1) * P],
                        xt[kt][:], start=(kt == 0), stop=(kt == 1))
                zt = s1.tile([P, MM], F16, tag="zt", name="zt", bufs=2)
                nc.scalar.activation(zt[:], pz[:], AF.Silu)
                nc.sync.dma_start(z_dr[m * P : (m + 1) * P, sl], zt[:])
            for m in range(DT):
                pu = ps1.tile([P, MM], F32, tag="ps1", name="ps1")
                for kt in range(DT):
                    nc.tensor.matmul(
                        pu[:], convT_sb[kt][:, m * P : (m + 1) * P],
                        xx_sb[kt][:], start=(kt == 0), stop=(kt == 3))
                nc.scalar.activation(
                    u_full[m][j][:], pu[:], AF.Silu,
                    bias=convb_sb[:, m : m + 1])

        # ---- stage 2: per token-chunk: k0 fwd scan + k1 bwd scan + merge ----
        # k=1 (the flipped direction) is computed as a BACKWARD scan in token
        # space: prep is pointwise (token-ordered, k=1 weights); only the
        # tensor_tensor_scan runs with reversed APs. Both yk come out
        # token-ordered, so the merge needs no flip and chunk ch pairs with
        # itself; stage-1 j-blocks interleave with the chunk loop.
        with tc.tile_pool(name="s2", bufs=2) as s2, \
             tc.tile_pool(name="bc", bufs=2) as bcp, \
             tc.tile_pool(name="acc", bufs=1) as accp, \
             tc.tile_pool(name="drm", bufs=2, space="DRAM") as drm, \
             tc.tile_pool(name="ps2", bufs=2, space="PSUM") as ps2:

            def pieces(lo, hi):
                o = lo
                while o < hi:
                    j = o // MM
                    e = min(hi, (j + 1) * MM)
                    yield o - lo, j, o - j * MM, e - o
                    o = e

            yk_tiles = {}
            preps = {}

            def chunk_range(k, ch):
                if k == 0:
                    t_lo = max(0, ch * TC - WU)
                    t_hi = (ch + 1) * TC
                    woff = ch * TC - t_lo          # good region starts here
                else:
                    t_lo = ch * TC
                    t_hi = min(L, (ch + 1) * TC + WU)
                    woff = 0                        # good region at the front
                return t_lo, t_hi, woff

            def prep_chunk(k, ch):
                # token range owned by this chunk + one-sided warmup
                t_lo, t_hi, woff = chunk_range(k, ch)
                TE = t_hi - t_lo

                # x_dbl -> dts, B, C rows (32-aligned PSUM partition blocks)
                xdb = s2.tile([96, TE], F16, tag="dts", name="dts")
                bc_stage = drm.tile([2 * N, TE], F16, tag="bcdram", name="bcdram")
                dts, bsb, csb = xdb[0:32], xdb[32:64], xdb[64:96]
                for do, j, so, w in pieces(t_lo, t_hi):
                    pxd = ps2.tile([96, MM], F32, tag="ps", name="ps")
                    for kt in range(DT):
                        nc.tensor.matmul(
                            pxd[:, :w], xprojT_sb[k, kt][:],
                            u_full[kt][j][:, so : so + w],
                            start=(kt == 0), stop=(kt == 3))
                    nc.scalar.copy(xdb[:, do : do + w], pxd[:, :w])
                nc.sync.dma_start(bc_stage[0:N, :], bsb[0:N, :])
                nc.sync.dma_start(bc_stage[N:, :], csb[0:N, :])

                # delta = softplus(dt_w @ dts + dt_b) via exp+ln; dtu = delta*u
                delta, dtu = [], []
                for m in range(DT):
                    dl = s2.tile([P, TE], F16, tag=f"delta{m}", name=f"delta{m}")
                    o = 0
                    while o < TE:
                        w = min(MM, TE - o)
                        pdt = ps2.tile([P, MM], F32, tag="ps", name="ps")
                        nc.tensor.matmul(
                            pdt[:, :w], dtwT_sb[k][:, m * P : (m + 1) * P],
                            dts[0:R, o : o + w], start=True, stop=True)
                        et = s2.tile([P, MM], F16, tag="et", name="et", bufs=1)
                        nc.scalar.activation(
                            et[:, :w], pdt[:, :w], AF.Exp,
                            bias=dtb_sb[k][:, m : m + 1])
                        nc.scalar.activation(
                            dl[:, o : o + w], et[:, :w], AF.Ln, bias=1.0)
                        o += w
                    du = s2.tile([P, TE], F16, tag=f"dtu{m}", name=f"dtu{m}")
                    for do, j, so, w in pieces(t_lo, t_hi):
                        nc.vector.tensor_mul(
                            du[:, do : do + w], dl[:, do : do + w],
                            u_full[m][j][:, so : so + w])
                    delta.append(dl)
                    dtu.append(du)

                # stage dtu + delta to DRAM for (n,dsub)-broadcast reads
                dtu_dram = drm.tile([DI, TE], F16, tag="dtudram", name="dtudram")
                dl_dram = drm.tile([DI, TE], F16, tag="dldram", name="dldram")
                for m in range(DT):
                    nc.sync.dma_start(dtu_dram[m * P : (m + 1) * P, :], dtu[m][:])
                    nc.sync.dma_start(dl_dram[m * P : (m + 1) * P, :], delta[m][:])

                # B/C rows broadcast into (n,dsub) layout: row n -> partitions
                # [8n, 8n+8)
                bbc = bcp.tile([P, TE], F16, tag="bbc", name="bbc")
                nc.sync.dma_start(
                    bbc[:],
                    bc_stage[0:N, :].unsqueeze(1).to_broadcast([N, 8, TE]))
                cbc = bcp.tile([P, TE], F16, tag="cbc", name="cbc")
                nc.sync.dma_start(
                    cbc[:],
                    bc_stage[N:, :].unsqueeze(1).to_broadcast([N, 8, TE]))
                preps[k, ch] = (dtu_dram, dl_dram, bbc, cbc)

            def scan_blocks(k, ch):
                t_lo, t_hi, woff = chunk_range(k, ch)
                TE = t_hi - t_lo
                dtu_dram, dl_dram, bbc, cbc = preps.pop((k, ch))

                # per d-block of 8 channels: partitions hold (n, dsub).
                # When the chunk has a warmup window (TE > TC), two d-blocks
                # are concatenated along the free axis into ONE scan: the
                # chain crossing from block i into block i+1 lands in i+1's
                # warmup region, which decays the contamination below fp16
                # resolution (>= 0.62 nats/step * 32 steps ~ e^-20) exactly
                # like the inter-chunk warmup.
                gsl = slice(woff, woff + TC)
                G = 2 if TE > TC else 1
                for B8 in range(DT):        # 128-d span
                    yps = [ps2.tile([P, MM], F32, tag="ypsum", name="ypsum",
                                    bufs=4) for _ in range(TC // MM)]
                    for lqg in range(16 // G):  # d-block group within span
                        b0 = B8 * 16 + G * lqg
                        # decay a = exp(A * delta), delta bcast from DRAM;
                        # dtu bcast on the SWDGE (Pool) queue to halve the
                        # sync-queue pressure
                        dlb = s2.tile([P, G * TE], F16, tag="dlb", name="dlb",
                                      bufs=2)
                        dbc = s2.tile([P, G * TE], F16, tag="dbc", name="dbc",
                                      bufs=2)
                        at = s2.tile([P, G * TE], F16, tag="a_t", name="a_t")
                        wt = s2.tile([P, G * TE], F16, tag="w_t", name="w_t")
                        for i in range(G):
                            b = b0 + i
                            isl = slice(i * TE, (i + 1) * TE)
                            nc.sync.dma_start(
                                dlb[:, isl],
                                dl_dram[b * 8 : b * 8 + 8, :].unsqueeze(0)
                                .to_broadcast([N, 8, TE]))
                            nc.scalar.activation(
                                at[:, isl], dlb[:, isl], AF.Exp,
                                scale=A_sb[k][:, b : b + 1])
                            nc.gpsimd.dma_start(
                                dbc[:, isl],
                                dtu_dram[b * 8 : b * 8 + 8, :].unsqueeze(0)
                                .to_broadcast([N, 8, TE]))
                            nc.vector.tensor_mul(wt[:, isl], dbc[:, isl], bbc[:])
                        ht = s2.tile([P, G * TE], F16, tag="h_t", name="h_t",
                                     bufs=1)
                        if k == 0:
                            nc.vector.tensor_tensor_scan(
                                ht[:], at[:], wt[:], 0.0, OP.mult, OP.add)
                        else:
                            nc.vector.tensor_tensor_scan(
                                ht[:, ::-1], at[:, ::-1], wt[:, ::-1], 0.0,
                                OP.mult, OP.add)
                        for i in range(G):
                            lq = G * lqg + i
                            yc = s2.tile([P, TC], F16, tag="yc", name="yc")
                            nc.vector.tensor_mul(
                                yc[:], ht[:, i * TE + woff : i * TE + woff + TC],
                                cbc[:, gsl])
                            # PE reduction over n into the d-major psum span
                            r, q = lq // 4, lq % 4
                            for j2 in range(TC // MM):
                                nc.tensor.matmul(
                                    yps[j2][32 * r : 32 * r + 32, :],
                                    l32_sb[:, 32 * q : 32 * q + 32],
                                    yc[:, j2 * MM : (j2 + 1) * MM],
                                    start=(q == 0), stop=False,
                                    tile_position=(0, 32 * r))
                    # y_k = Ds*u + ypsum: D-term is a diagonal matmul into
                    # the same PSUM accumulation; evac on ScalarE
                    yk = accp.tile([P, TC], F16, tag=f"yk{k}_{B8}",
                                   name=f"yk{k}_{B8}", bufs=2 if k == 0 else 1)
                    for do, j, so, w in pieces(ch * TC, (ch + 1) * TC):
                        nc.tensor.matmul(
                            yps[do // MM][:, :w], ldiag_sb[k, B8],
                            u_full[B8][j][:, so : so + w],
                            start=False, stop=True)
                        nc.scalar.copy(
                            yk[:, do : do + w],
                            yps[do // MM][:, :w])
                    yk_tiles[k, B8, ch] = yk

            def merge_chunk(ch):
                tok0 = ch * TC
                ymg = []
                for m in range(DT):
                    t = accp.tile([P, TC], F16, tag=f"ymg{m}", name=f"ymg{m}")
                    nc.vector.tensor_add(
                        t[:], yk_tiles.pop((0, m, ch))[:],
                        yk_tiles.pop((1, m, ch))[:])
                    ymg.append(t)
                o = 0
                while o < TC:
                    w = min(MM, TC - o)
                    pm = ps2.tile([1, MM], F32, tag="ps", name="pln")
                    for m in range(DT):
                        nc.tensor.matmul(
                            pm[:, :w], ones_sb[:], ymg[m][:, o : o + w],
                            start=(m == 0), stop=(m == 3))
                    pv = ps2.tile([1, MM], F32, tag="ps", name="pln")
                    sqs = []
                    for m in range(DT):
                        sq = s2.tile([P, MM], F16, tag="sq", name="sq", bufs=2)
                        nc.scalar.square(sq[:, :w], ymg[m][:, o : o + w])
                        sqs.append(sq)
                    for m in range(DT):
                        nc.tensor.matmul(
                            pv[:, :w], ones_sb[:], sqs[m][:, :w],
                            start=(m == 0), stop=(m == 3))
                    mu = s2.tile([1, MM], F16, tag="mu", name="mu", bufs=1)
                    nc.scalar.mul(mu[:, :w], pm[:, :w], 1.0 / DI)
                    msq = s2.tile([1, MM], F16, tag="msq", name="msq", bufs=1)
                    nc.scalar.mul(msq[:, :w], pv[:, :w], 1.0 / DI)
                    mu2 = s2.tile([1, MM], F16, tag="mu2", name="mu2", bufs=1)
                    nc.scalar.square(mu2[:, :w], mu[:, :w])
                    var = s2.tile([1, MM], F16, tag="var", name="var", bufs=1)
                    nc.vector.tensor_tensor(
                        var[:, :w], msq[:, :w], mu2[:, :w], OP.subtract)
                    lnv = s2.tile([1, MM], F32, tag="lnv", name="lnv", bufs=1)
                    nc.scalar.activation(lnv[:, :w], var[:, :w], AF.Ln,
                                         bias=eps_sb[0:1, :])
                    rstd = s2.tile([1, MM], F16, tag="rstd", name="rstd", bufs=1)
                    nc.scalar.activation(rstd[:, :w], lnv[:, :w], AF.Exp,
                                         scale=-0.5)
                    # partition-broadcast mu/rstd via PE ones-column matmul
                    # (avoids a DRAM round-trip on the critical path)
                    pbc = ps2.tile([P, MM], F32, tag="ypsum", name="lnbc", bufs=4)
                    nc.tensor.matmul(pbc[:, :w], onesr_sb[:], mu[:, :w],
                                     start=True, stop=True)
                    mubc = s2.tile([P, MM], F16, tag="mubc", name="mubc", bufs=1)
                    nc.scalar.copy(mubc[:, :w], pbc[:, :w])
                    pbc2 = ps2.tile([P, MM], F32, tag="ypsum", name="lnbc", bufs=4)
                    nc.tensor.matmul(pbc2[:, :w], onesr_sb[:], rstd[:, :w],
                                     start=True, stop=True)
                    rsbc = s2.tile([P, MM], F16, tag="rsbc", name="rsbc", bufs=1)
                    nc.scalar.copy(rsbc[:, :w], pbc2[:, :w])
                    yzs = []
                    for m in range(DT):
                        t1 = s2.tile([P, MM], F16, tag="t1", name="t1", bufs=1)
                        nc.vector.tensor_tensor(
                            t1[:, :w], ymg[m][:, o : o + w], mubc[:, :w],
                            OP.subtract)
                        t2 = s2.tile([P, MM], F16, tag="t2", name="t2", bufs=1)
                        nc.vector.tensor_mul(t2[:, :w], t1[:, :w], rsbc[:, :w])
                        t3 = s2.tile([P, MM], F16, tag="t3", name="t3", bufs=2)
                        nc.scalar.activation(
                            t3[:, :w], t2[:, :w], AF.Identity,
                            bias=lnb_sb[:, m : m + 1], scale=lnw_sb[:, m : m + 1])
                        zt2 = s2.tile([P, MM], F16, tag="zt2", name="zt2", bufs=1)
                        nc.sync.dma_start(
                            zt2[:, :w],
                            z_dr[m * P : (m + 1) * P, tok0 + o : tok0 + o + w])
                        yz = s2.tile([P, MM], F16, tag=f"yz{m}", name=f"yz{m}",
                                     bufs=1)
                        nc.vector.tensor_mul(yz[:, :w], t3[:, :w], zt2[:, :w])
                        yzs.append(yz)
                    for mo in range(DIM // P):
                        po = ps2.tile([P, MM], F32, tag="ps", name="ps")
                        for kt in range(DT):
                            nc.tensor.matmul(
                                po[:, :w], woutT_sb[kt][:, mo * P : (mo + 1) * P],
                                yzs[kt][:, :w], start=(kt == 0), stop=(kt == 3))
                        osb = s2.tile([P, MM], F32, tag=f"osb{mo}", name=f"osb{mo}", bufs=1)
                        nc.scalar.copy(osb[:, :w], po[:, :w])
                        nc.sync.dma_start(
                            out_dr[mo * P : (mo + 1) * P, tok0 + o : tok0 + o + w],
                            osb[:, :w])
                    o += w

            # software pipeline: preps (PE/ACT/DMA-heavy) are emitted ahead
            # of the DVE-heavy scan block loops so every engine always has
            # ready work queued; stage-1 j-pairs stay adjacent to limit
            # silu<->exp activation-table reloads.
            # u-deps: prep(1,ch) needs u to (ch+1)*TC+WU -> j=2ch+2;
            #         prep(0,ch+1) needs u to (ch+2)*TC  -> j=2ch+3.
            stage1_j(0)
            stage1_j(1)
            prep_chunk(0, 0)
            for ch in range(NCH):
                if ch + 1 < NCH:
                    stage1_j(2 * ch + 2)
                    stage1_j(2 * ch + 3)
                prep_chunk(1, ch)
                scan_blocks(0, ch)
                if ch + 1 < NCH:
                    prep_chunk(0, ch + 1)
                scan_blocks(1, ch)
                merge_chunk(ch)

        ps1.release()
        s1.release()
        zdrp.release()
        upool.release()
        cpool.release()

    _CACHE["nc"] = nc
    return nc


# ------------------------------------------------------------------ kernel --
def kernel(**inputs):
    x = np.asarray(inputs["x"], np.float32)
    W_in = np.asarray(inputs["W_in"], np.float32)
    conv_w = np.asarray(inputs["conv_w"], np.float32)
    conv_b = np.asarray(inputs["conv_b"], np.float32)
    x_proj_w = np.asarray(inputs["x_proj_w"], np.float32)
    dt_w = np.asarray(inputs["dt_w"], np.float32)
    dt_b = np.asarray(inputs["dt_b"], np.float32)
    A_logs = np.asarray(inputs["A_logs"], np.float32)
    Ds = np.asarray(inputs["Ds"], np.float32)
    ln_w = np.asarray(inputs["ln_w"], np.float32)
    ln_b = np.asarray(inputs["ln_b"], np.float32)
    W_out = np.asarray(inputs["W_out"], np.float32)

    nc = _build()
    from concourse.bass_utils import run_bass_kernel_spmd

    def pack_cols(v):  # [K?, DI] -> [P, DT] column per d-tile
        return np.ascontiguousarray(v.reshape(DT, P).T)

    def _pad_xproj(xpw):
        t = xpw.transpose(0, 2, 1)                              # [K, DI, 48]
        out = np.zeros((K, DI, 96), np.float16)
        out[:, :, 0:16] = t[:, :, 0:16]
        out[:, :, 32:48] = t[:, :, 16:32]
        out[:, :, 64:80] = t[:, :, 32:48]
        return out

    xTb = np.ascontiguousarray(x.transpose(0, 2, 1))           # [B, DIM, L]
    A = (-np.exp(A_logs)).reshape(K, DI, N)
    A_pack = np.stack([
        np.ascontiguousarray(
            A[k].reshape(64, 8, N).transpose(2, 1, 0).reshape(P, 64))
        for k in range(K)])
    l8 = np.zeros((8, P), np.float16)
    for p in range(P):
        l8[p % 8, p] = 1.0
    # variant q (cols [32q,32q+32)): one-hot at local col 8q + p%8
    l32 = np.zeros((P, P), np.float16)
    for q in range(4):
        for p in range(P):
            l32[p, 32 * q + 8 * q + (p % 8)] = 1.0
    Ds2 = Ds.reshape(K, DI)
    ldiag = np.zeros((K, DT, P, P), np.float16)
    for k in range(K):
        for sp in range(DT):
            np.fill_diagonal(ldiag[k, sp], Ds2[k, sp * P : (sp + 1) * P])
    shared = {
        "winT": np.ascontiguousarray(W_in.T).astype(np.float16),
        "convT": np.ascontiguousarray(conv_w.T).astype(np.float16),
        "convb": pack_cols(conv_b),
        "nconvb": pack_cols(-conv_b),
        "xprojT": _pad_xproj(x_proj_w),                     # [K, DI, 96]
        "dtwT": np.ascontiguousarray(
            dt_w.transpose(0, 2, 1)).astype(np.float16),        # [K, R, DI]
        "dtb": np.stack([pack_cols(dt_b[k]) for k in range(K)]),
        "A": A_pack,
        "Ds": np.stack([pack_cols(Ds.reshape(K, DI)[k]) for k in range(K)]),
        "lnw": pack_cols(ln_w),
        "lnb": pack_cols(ln_b),
        "woutT": np.ascontiguousarray(W_out.T).astype(np.float16),  # [DI, DIM]
        "ones": np.ones((P, 1), np.float16),
        "onesr": np.ones((1, P), np.float16),
        "eps": np.full((P, 1), 1e-5, np.float32),
        "l8": l8,
        "l32": l32,
        "ldiag": ldiag,
    }
    in_maps = [dict(shared, xT=np.ascontiguousarray(xTb[b]).astype(np.float16))
               for b in range(B)]

    trace = _install_profile_shim()
    try:
        res = run_bass_kernel_spmd(
            nc, in_maps, core_ids=list(range(B)), trace=trace)
    except Exception:
        if not trace:
            raise
        res = run_bass_kernel_spmd(
            nc, in_maps, core_ids=list(range(B)), trace=False)
    if res.exec_time_ns is not None:
        print(f"HW exec time: {res.exec_time_ns} ns")

    out = np.stack([res.results[b]["out"].T for b in range(B)])  # [B, L, DIM]
    return out.astype(np.float32)


if __name__ == "__main__":
    nc = _build()
    n_inst = sum(
        len(blk.instructions) for fn in nc.m.functions for blk in fn.blocks)
    print("built ok, instructions:", n_inst)



# revision 25
# speedup vs baseline: 1.0186x; 1.0186x over previous
"""Trainium2 Bass kernel for the Mamba-style K=2 selective-scan block.

Strategy: data-parallel over batch B=8 across the 8 NeuronCores. Per core,
everything runs in [channel, time] layout:
  - in_proj / 1x1 conv / SiLU on TensorE+ScalarE (fp32 matmuls)
  - selective scan: per (direction k, d-tile, state n): decay a = exp(A*delta)
    on ScalarE (per-partition scale), w = dtu*B_bc and the affine recurrence
    (tensor_tensor_scan) + y accumulation on VectorE, all fp16
  - time is chunked (TC=1024) with a 32-step warmup window; the per-step decay
    is >= ~0.6 nats so 32 steps of warmup reproduce the exact fp32 state,
    making chunks independent (no cross-chunk scan state)
  - B/C rows are broadcast across partitions via DRAM round-trip DMA
  - merge fwd/rev + LayerNorm + z-gate + out_proj on PE/ACT/DVE
"""
import numpy as np

B, S, DIM = 8, 4096, 256
DI, K, N, R = 512, 2, 16, 16
L = S
P = 128
DT = DI // P          # 4 d-tiles of 128 channels
TC = 1024             # output tokens per chunk
WU = 32               # warmup steps (decay >= 0.6 nats/step -> exp(-19) left)
NCH = L // TC
MM = 512              # matmul free-dim chunk

_CACHE = {}


# ----------------------------------------------------------------- patches --
def _install_patches():
    """Walrus in this container rejects >1 semaphore wait on some instruction
    structs; hoist surplus waits onto InstNoOp carriers. Also shim
    antenv.axon_hooks so trace=True can NTFF-profile (optional)."""
    if _CACHE.get("patched"):
        return
    import concourse.tile as tile
    from concourse import mybir

    MAXW = 1
    orig_commit = tile.TileContext._commit_instruction

    def split_waits(self, inst):
        si = inst.sync_info
        if si is None or not si.on_wait or len(si.on_wait) <= MAXW:
            return
        waits = list(si.on_wait)
        surplus, keep = waits[:-MAXW], waits[-MAXW:]
        eng = inst.engine
        if eng == mybir.EngineType.Unassigned:
            return
        for i in range(0, len(surplus), MAXW):
            nop = mybir.InstNoOp(name=f"{inst.name}-w{i}", ins=[], outs=[])
            nop.engine = eng
            nop.sync_info = mybir.SyncInfo(
                on_wait=surplus[i : i + MAXW], on_update=[]
            )
            self._add_instruction(nop)
        inst.sync_info = mybir.SyncInfo(on_wait=keep, on_update=list(si.on_update))

    def patched_commit(self, inst, lazy_reg_writes=True):
        split_waits(self, inst)
        return orig_commit(self, inst, lazy_reg_writes)

    def patched_drain_and_barrier(self, tick_clock, wait_clock):
        from concourse.tile import ScopedClock

        drain_inst = self.nc.sync.drain()
        wait_clock.add_sem_waits(
            drain_inst.ins, ScopedClock({None: tick_clock.global_clock})
        )
        si = drain_inst.ins.sync_info
        waits = list(si.on_wait) if si is not None and si.on_wait else []
        if len(waits) > MAXW:
            drain_inst.ins.sync_info = mybir.SyncInfo(
                on_wait=waits[:MAXW], on_update=list(si.on_update)
            )
            rest = waits[MAXW:]
            for i in range(0, len(rest), MAXW):
                extra = self.nc.sync.drain()
                extra.ins.sync_info = mybir.SyncInfo(
                    on_wait=rest[i : i + MAXW], on_update=[]
                )
        self.nc.all_engine_barrier()
        assert self.sems is not None
        popped = self.nc._tile_sem_poison_stack.pop()
        assert popped is self._sem_poison
        self.nc.clear_and_free_semaphores(list(self.sems.allocated().values()))
        self.nc.all_engine_barrier()

    tile.TileContext._commit_instruction = patched_commit
    tile.TileContext._drain_and_barrier = patched_drain_and_barrier
    _CACHE["patched"] = True


def _install_profile_shim():
    if _CACHE.get("shim"):
        return True
    try:
        import sys, types
        import antenv

        name = "antenv.axon_hooks"
        if name not in sys.modules:
            mod = types.ModuleType(name)
            hook = [None]
            mod.set_axon_ntff_profile_hook = lambda h: hook.__setitem__(0, h)
            mod.get_axon_ntff_profile_hook = lambda: hook[0]
            sys.modules[name] = mod
            antenv.axon_hooks = mod
            from trn_agent_boot.trn_boot import _ntff_profile_via_ctypes

            h = _ntff_profile_via_ctypes("/opt/axon/libaxon_pjrt.so")
            if h is not None:
                mod.set_axon_ntff_profile_hook(h)
        _CACHE["shim"] = True
        return True
    except Exception:
        return False


# ------------------------------------------------------------------- build --
def _build():
    if "nc" in _CACHE:
        return _CACHE["nc"]
    _install_patches()
    import concourse.bass as bass
    import concourse.tile as tile
    from concourse import mybir

    F32 = mybir.dt.float32
    F16 = mybir.dt.float16
    AF = mybir.ActivationFunctionType
    OP = mybir.AluOpType

    nc = bass.Bass("TRN2", target_bir_lowering=False, debug=False, num_devices=1)

    def din(name, shape, dt=F32):
        return nc.dram_tensor(name, shape, dt, kind="ExternalInput").ap()

    xT = din("xT", [DIM, L], F16)
    winT = din("winT", [DIM, 2 * DI], F16)
    convT = din("convT", [DI, DI], F16)
    convb = din("convb", [P, DT])
    nconvbd = din("nconvb", [P, DT])
    xprojT = din("xprojT", [K, DI, 96], F16)
    dtwT = din("dtwT", [K, R, DI], F16)
    dtb = din("dtb", [K, P, DT])
    Adr = din("A", [K, P, 64])
    Dsdr = din("Ds", [K, P, DT])
    lnw = din("lnw", [P, DT])
    lnb = din("lnb", [P, DT])
    woutT = din("woutT", [DI, DIM], F16)
    ones = din("ones", [P, 1], F16)
    onesr = din("onesr", [1, P], F16)
    l8d = din("l8", [8, P], F16)
    l32d = din("l32", [P, P], F16)
    ldg = din("ldiag", [K, DT, P, P], F16)
    epsd = din("eps", [P, 1])
    out_dr = nc.dram_tensor("out", [DIM, L], F32, kind="ExternalOutput").ap()

    with tile.TileContext(nc) as tc:
        cpool = tc.alloc_tile_pool(name="const", bufs=1)
        upool = tc.alloc_tile_pool(name="ufull", bufs=1)

        # ---- constants to SBUF ----
        winT_sb = []
        for kt in range(2):
            t = cpool.tile([P, 2 * DI], F16, tag=f"winT{kt}", name=f"winT{kt}")
            nc.sync.dma_start(t[:], winT[kt * P : (kt + 1) * P, :])
            winT_sb.append(t)
        convT_sb = []
        for kt in range(DT):
            t = cpool.tile([P, DI], F16, tag=f"convT{kt}", name=f"convT{kt}")
            nc.sync.dma_start(t[:], convT[kt * P : (kt + 1) * P, :])
            convT_sb.append(t)
        xprojT_sb = {}
        for k in range(K):
            for kt in range(DT):
                t = cpool.tile([P, 96], F16, tag=f"xprojT{k}{kt}", name=f"xprojT{k}{kt}")
                nc.sync.dma_start(t[:], xprojT[k, kt * P : (kt + 1) * P, :])
                xprojT_sb[k, kt] = t
        dtwT_sb = {}
        for k in range(K):
            t = cpool.tile([R, DI], F16, tag=f"dtwT{k}", name=f"dtwT{k}")
            nc.sync.dma_start(t[:], dtwT[k])
            dtwT_sb[k] = t
        woutT_sb = []
        for kt in range(DT):
            t = cpool.tile([P, DIM], F16, tag=f"woutT{kt}", name=f"woutT{kt}")
            nc.sync.dma_start(t[:], woutT[kt * P : (kt + 1) * P, :])
            woutT_sb.append(t)
        A_sb, dtb_sb, Ds_sb = {}, {}, {}
        for k in range(K):
            t = cpool.tile([P, 64], F32, tag=f"A{k}", name=f"A{k}")
            nc.sync.dma_start(t[:], Adr[k])
            A_sb[k] = t
            t = cpool.tile([P, DT], F32, tag=f"dtb{k}", name=f"dtb{k}")
            nc.sync.dma_start(t[:], dtb[k])
            dtb_sb[k] = t
            t = cpool.tile([P, DT], F32, tag=f"Ds{k}", name=f"Ds{k}")
            nc.sync.dma_start(t[:], Dsdr[k])
            Ds_sb[k] = t
        convb_sb = cpool.tile([P, DT], F32, tag="convb", name="convb")
        nc.sync.dma_start(convb_sb[:], convb[:])
        nconvb_sb = cpool.tile([P, DT], F32, tag="nconvb", name="nconvb")
        nc.sync.dma_start(nconvb_sb[:], nconvbd[:])
        lnw_sb = cpool.tile([P, DT], F32, tag="lnw", name="lnw")
        nc.sync.dma_start(lnw_sb[:], lnw[:])
        lnb_sb = cpool.tile([P, DT], F32, tag="lnb", name="lnb")
        nc.sync.dma_start(lnb_sb[:], lnb[:])
        ones_sb = cpool.tile([P, 1], F16, tag="ones", name="ones")
        nc.sync.dma_start(ones_sb[:], ones[:])
        onesr_sb = cpool.tile([1, P], F16, tag="onesr", name="onesr")
        nc.sync.dma_start(onesr_sb[:], onesr[:])
        eps_sb = cpool.tile([P, 1], F32, tag="eps", name="eps")
        nc.sync.dma_start(eps_sb[:], epsd[:])
        l8_sb = cpool.tile([8, P], F16, tag="l8", name="l8")
        nc.sync.dma_start(l8_sb[:], l8d[:])
        l32_sb = cpool.tile([P, P], F16, tag="l32", name="l32")
        nc.sync.dma_start(l32_sb[:], l32d[:])
        ldiag_sb = {}
        for k in range(K):
            for sp in range(DT):
                t = cpool.tile([P, P], F16, tag=f"ldg{k}{sp}", name=f"ldg{k}{sp}")
                nc.sync.dma_start(t[:], ldg[k, sp])
                ldiag_sb[k, sp] = t

        # persistent activations; z spills to DRAM (read back at merge);
        # u is split into per-512-column tiles for fine-grained deps
        NJ = L // MM
        u_full = [[upool.tile([P, MM], F16, tag=f"u{m}_{j}", name=f"u{m}_{j}")
                   for j in range(NJ)] for m in range(DT)]
        zdrp = tc.alloc_tile_pool(name="zdr", bufs=1, space="DRAM")
        z_dr = zdrp.tile([DI, L], F16, tag="zdr", name="zdr")

        # ---- stage 1: in_proj -> silu(z); conv1x1 -> silu -> u (per j) ----
        s1 = tc.alloc_tile_pool(name="s1", bufs=2)
        ps1 = tc.alloc_tile_pool(name="ps1", bufs=2, space="PSUM")

        def stage1_j(j):
            sl = slice(j * MM, (j + 1) * MM)
            xt = []
            for kt in range(2):
                # SWDGE queue: x loads bypass the constant-load serialization
                # on the sync queue at startup
                t = s1.tile([P, MM], F16, tag=f"xt{kt}", name=f"xt{kt}")
                nc.gpsimd.dma_start(t[:], xT[kt * P : (kt + 1) * P, sl])
                xt.append(t)
            xx_sb = []
            for m in range(DT):
                pz = ps1.tile([P, MM], F32, tag="ps1", name="ps1")
                for kt in range(2):
                    nc.tensor.matmul(
                        pz[:], winT_sb[kt][:, m * P : (m + 1) * P], xt[kt][:],
                        start=(kt == 0), stop=(kt == 1))
                t = s1.tile([P, MM], F16, tag=f"xx{m}", name=f"xx{m}")
                nc.scalar.copy(t[:], pz[:])
                xx_sb.append(t)
            for m in range(DT):
                pz = ps1.tile([P, MM], F32, tag="ps1", name="ps1")
                for kt in range(2):
                    nc.tensor.matmul(
                        pz[:], winT_sb[kt][:, DI + m * P : DI + (m + 1) * P],
                        xt[kt][:], start=(kt == 0), stop=(kt == 1))
                zt = s1.tile([P, MM], F16, tag="zt", name="zt", bufs=2)
                nc.scalar.activation(zt[:], pz[:], AF.Silu)
                nc.sync.dma_start(z_dr[m * P : (m + 1) * P, sl], zt[:])
            for m in range(DT):
                pu = ps1.tile([P, MM], F32, tag="ps1", name="ps1")
                for kt in range(DT):
                    nc.tensor.matmul(
                        pu[:], convT_sb[kt][:, m * P : (m + 1) * P],
                        xx_sb[kt][:], start=(kt == 0), stop=(kt == 3))
                nc.scalar.activation(
                    u_full[m][j][:], pu[:], AF.Silu,
                    bias=convb_sb[:, m : m + 1])

        # ---- stage 2: per token-chunk: k0 fwd scan + k1 bwd scan + merge ----
        # k=1 (the flipped direction) is computed as a BACKWARD scan in token
        # space: prep is pointwise (token-ordered, k=1 weights); only the
        # tensor_tensor_scan runs with reversed APs. Both yk come out
        # token-ordered, so the merge needs no flip and chunk ch pairs with
        # itself; stage-1 j-blocks interleave with the chunk loop.
        with tc.tile_pool(name="s2", bufs=2) as s2, \
             tc.tile_pool(name="bc", bufs=2) as bcp, \
             tc.tile_pool(name="acc", bufs=1) as accp, \
             tc.tile_pool(name="drm", bufs=2, space="DRAM") as drm, \
             tc.tile_pool(name="ps2", bufs=2, space="PSUM") as ps2:

            def pieces(lo, hi):
                o = lo
                while o < hi:
                    j = o // MM
                    e = min(hi, (j + 1) * MM)
                    yield o - lo, j, o - j * MM, e - o
                    o = e

            yk_tiles = {}
            preps = {}

            def chunk_range(k, ch):
                if k == 0:
                    t_lo = max(0, ch * TC - WU)
                    t_hi = (ch + 1) * TC
                    woff = ch * TC - t_lo          # good region starts here
                else:
                    t_lo = ch * TC
                    t_hi = min(L, (ch + 1) * TC + WU)
                    woff = 0                        # good region at the front
                return t_lo, t_hi, woff

            def prep_chunk(k, ch):
                # token range owned by this chunk + one-sided warmup
                t_lo, t_hi, woff = chunk_range(k, ch)
                TE = t_hi - t_lo

                # x_dbl -> dts, B, C rows (32-aligned PSUM partition blocks)
                xdb = s2.tile([96, TE], F16, tag="dts", name="dts")
                bc_stage = drm.tile([2 * N, TE], F16, tag="bcdram", name="bcdram")
                dts, bsb, csb = xdb[0:32], xdb[32:64], xdb[64:96]
                for do, j, so, w in pieces(t_lo, t_hi):
                    pxd = ps2.tile([96, MM], F32, tag="ps", name="ps")
                    for kt in range(DT):
                        nc.tensor.matmul(
                            pxd[:, :w], xprojT_sb[k, kt][:],
                            u_full[kt][j][:, so : so + w],
                            start=(kt == 0), stop=(kt == 3))
                    nc.scalar.copy(xdb[:, do : do + w], pxd[:, :w])
                nc.sync.dma_start(bc_stage[0:N, :], bsb[0:N, :])
                nc.sync.dma_start(bc_stage[N:, :], csb[0:N, :])

                # delta = softplus(dt_w @ dts + dt_b) via exp+ln; dtu = delta*u
                delta, dtu = [], []
                for m in range(DT):
                    dl = s2.tile([P, TE], F16, tag=f"delta{m}", name=f"delta{m}")
                    o = 0
                    while o < TE:
                        w = min(MM, TE - o)
                        pdt = ps2.tile([P, MM], F32, tag="ps", name="ps")
                        nc.tensor.matmul(
                            pdt[:, :w], dtwT_sb[k][:, m * P : (m + 1) * P],
                            dts[0:R, o : o + w], start=True, stop=True)
                        et = s2.tile([P, MM], F16, tag="et", name="et", bufs=1)
                        nc.scalar.activation(
                            et[:, :w], pdt[:, :w], AF.Exp,
                            bias=dtb_sb[k][:, m : m + 1])
                        nc.scalar.activation(
                            dl[:, o : o + w], et[:, :w], AF.Ln, bias=1.0)
                        o += w
                    du = s2.tile([P, TE], F16, tag=f"dtu{m}", name=f"dtu{m}")
                    for do, j, so, w in pieces(t_lo, t_hi):
                        nc.vector.tensor_mul(
                            du[:, do : do + w], dl[:, do : do + w],
                            u_full[m][j][:, so : so + w])
                    delta.append(dl)
                    dtu.append(du)

                # stage dtu + delta to DRAM for (n,dsub)-broadcast reads
                dtu_dram = drm.tile([DI, TE], F16, tag="dtudram", name="dtudram")
                dl_dram = drm.tile([DI, TE], F16, tag="dldram", name="dldram")
                for m in range(DT):
                    nc.sync.dma_start(dtu_dram[m * P : (m + 1) * P, :], dtu[m][:])
                    nc.sync.dma_start(dl_dram[m * P : (m + 1) * P, :], delta[m][:])

                # B/C rows broadcast into (n,dsub) layout: row n -> partitions
                # [8n, 8n+8)
                bbc = bcp.tile([P, TE], F16, tag="bbc", name="bbc")
                nc.sync.dma_start(
                    bbc[:],
                    bc_stage[0:N, :].unsqueeze(1).to_broadcast([N, 8, TE]))
                cbc = bcp.tile([P, TE], F16, tag="cbc", name="cbc")
                nc.sync.dma_start(
                    cbc[:],
                    bc_stage[N:, :].unsqueeze(1).to_broadcast([N, 8, TE]))
                preps[k, ch] = (dtu_dram, dl_dram, bbc, cbc)

            def scan_blocks(k, ch):
                t_lo, t_hi, woff = chunk_range(k, ch)
                TE = t_hi - t_lo
                dtu_dram, dl_dram, bbc, cbc = preps.pop((k, ch))

                # per d-block of 8 channels: partitions hold (n, dsub).
                # When the chunk has a warmup window (TE > TC), two d-blocks
                # are concatenated along the free axis into ONE scan: the
                # chain crossing from block i into block i+1 lands in i+1's
                # warmup region, which decays the contamination below fp16
                # resolution (>= 0.62 nats/step * 32 steps ~ e^-20) exactly
                # like the inter-chunk warmup.
                gsl = slice(woff, woff + TC)
                G = 2 if TE > TC else 1
                for B8 in range(DT):        # 128-d span
                    yps = [ps2.tile([P, MM], F32, tag="ypsum", name="ypsum",
                                    bufs=4) for _ in range(TC // MM)]
                    for lqg in range(16 // G):  # d-block group within span
                        b0 = B8 * 16 + G * lqg
                        # decay a = exp(A * delta), delta bcast from DRAM;
                        # dtu bcast on the SWDGE (Pool) queue to halve the
                        # sync-queue pressure
                        dlb = s2.tile([P, G * TE], F16, tag="dlb", name="dlb",
                                      bufs=2)
                        dbc = s2.tile([P, G * TE], F16, tag="dbc", name="dbc",
                                      bufs=2)
                        at = s2.tile([P, G * TE], F16, tag="a_t", name="a_t")
                        wt = s2.tile([P, G * TE], F16, tag="w_t", name="w_t")
                        for i in range(G):
                            b = b0 + i
                            isl = slice(i * TE, (i + 1) * TE)
                            nc.sync.dma_start(
                                dlb[:, isl],
                                dl_dram[b * 8 : b * 8 + 8, :].unsqueeze(0)
                                .to_broadcast([N, 8, TE]))
                            nc.scalar.activation(
                                at[:, isl], dlb[:, isl], AF.Exp,
                                scale=A_sb[k][:, b : b + 1])
                            nc.gpsimd.dma_start(
                                dbc[:, isl],
                                dtu_dram[b * 8 : b * 8 + 8, :].unsqueeze(0)
                                .to_broadcast([N, 8, TE]))
                            nc.vector.tensor_mul(wt[:, isl], dbc[:, isl], bbc[:])
                        ht = s2.tile([P, G * TE], F16, tag="h_t", name="h_t",
                                     bufs=1)
                        if k == 0:
                            nc.vector.tensor_tensor_scan(
                                ht[:], at[:], wt[:], 0.0, OP.mult, OP.add)
                        else:
                            nc.vector.tensor_tensor_scan(
                                ht[:, ::-1], at[:, ::-1], wt[:, ::-1], 0.0,
                                OP.mult, OP.add)
                        for i in range(G):
                            lq = G * lqg + i
                            yc = s2.tile([P, TC], F16, tag="yc", name="yc")
                            nc.vector.tensor_mul(
                                yc[:], ht[:, i * TE + woff : i * TE + woff + TC],
                                cbc[:, gsl])
                            # PE reduction over n into the d-major psum span
                            r, q = lq // 4, lq % 4
                            for j2 in range(TC // MM):
                                nc.tensor.matmul(
                                    yps[j2][32 * r : 32 * r + 32, :],
                                    l32_sb[:, 32 * q : 32 * q + 32],
                                    yc[:, j2 * MM : (j2 + 1) * MM],
                                    start=(q == 0), stop=False,
                                    tile_position=(0, 32 * r))
                    # y_k = Ds*u + ypsum: D-term is a diagonal matmul into
                    # the same PSUM accumulation; evac on ScalarE
                    yk = accp.tile([P, TC], F16, tag=f"yk{k}_{B8}",
                                   name=f"yk{k}_{B8}", bufs=1)
                    for do, j, so, w in pieces(ch * TC, (ch + 1) * TC):
                        nc.tensor.matmul(
                            yps[do // MM][:, :w], ldiag_sb[k, B8],
                            u_full[B8][j][:, so : so + w],
                            start=False, stop=True)
                        nc.scalar.copy(
                            yk[:, do : do + w],
                            yps[do // MM][:, :w])
                    yk_tiles[k, B8, ch] = yk

            def merge_chunk(ch):
                tok0 = ch * TC
                ymg = []
                for m in range(DT):
                    t = accp.tile([P, TC], F16, tag=f"ymg{m}", name=f"ymg{m}")
                    nc.vector.tensor_add(
                        t[:], yk_tiles.pop((0, m, ch))[:],
                        yk_tiles.pop((1, m, ch))[:])
                    ymg.append(t)
                o = 0
                while o < TC:
                    w = min(MM, TC - o)
                    pm = ps2.tile([1, MM], F32, tag="ps", name="pln")
                    for m in range(DT):
                        nc.tensor.matmul(
                            pm[:, :w], ones_sb[:], ymg[m][:, o : o + w],
                            start=(m == 0), stop=(m == 3))
                    pv = ps2.tile([1, MM], F32, tag="ps", name="pln")
                    sqs = []
                    for m in range(DT):
                        sq = s2.tile([P, MM], F16, tag="sq", name="sq", bufs=2)
                        nc.scalar.square(sq[:, :w], ymg[m][:, o : o + w])
                        sqs.append(sq)
                    for m in range(DT):
                        nc.tensor.matmul(
                            pv[:, :w], ones_sb[:], sqs[m][:, :w],
                            start=(m == 0), stop=(m == 3))
                    mu = s2.tile([1, MM], F16, tag="mu", name="mu", bufs=1)
                    nc.scalar.mul(mu[:, :w], pm[:, :w], 1.0 / DI)
                    msq = s2.tile([1, MM], F16, tag="msq", name="msq", bufs=1)
                    nc.scalar.mul(msq[:, :w], pv[:, :w], 1.0 / DI)
                    mu2 = s2.tile([1, MM], F16, tag="mu2", name="mu2", bufs=1)
                    nc.scalar.square(mu2[:, :w], mu[:, :w])
                    var = s2.tile([1, MM], F16, tag="var", name="var", bufs=1)
                    nc.vector.tensor_tensor(
                        var[:, :w], msq[:, :w], mu2[:, :w], OP.subtract)
                    lnv = s2.tile([1, MM], F32, tag="lnv", name="lnv", bufs=1)
                    nc.scalar.activation(lnv[:, :w], var[:, :w], AF.Ln,
                                         bias=eps_sb[0:1, :])
                    rstd = s2.tile([1, MM], F16, tag="rstd", name="rstd", bufs=1)
                    nc.scalar.activation(rstd[:, :w], lnv[:, :w], AF.Exp,
                                         scale=-0.5)
                    # partition-broadcast mu/rstd via PE ones-column matmul
                    # (avoids a DRAM round-trip on the critical path)
                    pbc = ps2.tile([P, MM], F32, tag="ypsum", name="lnbc", bufs=4)
                    nc.tensor.matmul(pbc[:, :w], onesr_sb[:], mu[:, :w],
                                     start=True, stop=True)
                    mubc = s2.tile([P, MM], F16, tag="mubc", name="mubc", bufs=1)
                    nc.scalar.copy(mubc[:, :w], pbc[:, :w])
                    pbc2 = ps2.tile([P, MM], F32, tag="ypsum", name="lnbc", bufs=4)
                    nc.tensor.matmul(pbc2[:, :w], onesr_sb[:], rstd[:, :w],
                                     start=True, stop=True)
                    rsbc = s2.tile([P, MM], F16, tag="rsbc", name="rsbc", bufs=1)
                    nc.scalar.copy(rsbc[:, :w], pbc2[:, :w])
                    yzs = []
                    for m in range(DT):
                        t1 = s2.tile([P, MM], F16, tag="t1", name="t1", bufs=2)
                        nc.vector.tensor_tensor(
                            t1[:, :w], ymg[m][:, o : o + w], mubc[:, :w],
                            OP.subtract)
                        t2 = s2.tile([P, MM], F16, tag="t2", name="t2", bufs=2)
                        nc.vector.tensor_mul(t2[:, :w], t1[:, :w], rsbc[:, :w])
                        t3 = s2.tile([P, MM], F16, tag="t3", name="t3", bufs=2)
                        nc.scalar.activation(
                            t3[:, :w], t2[:, :w], AF.Identity,
                            bias=lnb_sb[:, m : m + 1], scale=lnw_sb[:, m : m + 1])
                        zt2 = s2.tile([P, MM], F16, tag="zt2", name="zt2", bufs=2)
                        nc.sync.dma_start(
                            zt2[:, :w],
                            z_dr[m * P : (m + 1) * P, tok0 + o : tok0 + o + w])
                        yz = s2.tile([P, MM], F16, tag=f"yz{m}", name=f"yz{m}",
                                     bufs=1)
                        nc.vector.tensor_mul(yz[:, :w], t3[:, :w], zt2[:, :w])
                        yzs.append(yz)
                    for mo in range(DIM // P):
                        po = ps2.tile([P, MM], F32, tag="ps", name="ps")
                        for kt in range(DT):
                            nc.tensor.matmul(
                                po[:, :w], woutT_sb[kt][:, mo * P : (mo + 1) * P],
                                yzs[kt][:, :w], start=(kt == 0), stop=(kt == 3))
                        osb = s2.tile([P, MM], F32, tag=f"osb{mo}", name=f"osb{mo}", bufs=1)
                        nc.scalar.copy(osb[:, :w], po[:, :w])
                        nc.sync.dma_start(
                            out_dr[mo * P : (mo + 1) * P, tok0 + o : tok0 + o + w],
                            osb[:, :w])
                    o += w

            # software pipeline: preps (PE/ACT/DMA-heavy) are emitted ahead
            # of the DVE-heavy scan block loops so every engine always has
            # ready work queued; stage-1 j-pairs stay adjacent to limit
            # silu<->exp activation-table reloads.
            # u-deps: prep(1,ch) needs u to (ch+1)*TC+WU -> j=2ch+2;
            #         prep(0,ch+1) needs u to (ch+2)*TC  -> j=2ch+3.
            stage1_j(0)
            stage1_j(1)
            prep_chunk(0, 0)
            for ch in range(NCH):
                if ch + 1 < NCH:
                    stage1_j(2 * ch + 2)
                    stage1_j(2 * ch + 3)
                prep_chunk(1, ch)
                scan_blocks(0, ch)
                if ch + 1 < NCH:
                    prep_chunk(0, ch + 1)
                scan_blocks(1, ch)
                merge_chunk(ch)

        ps1.release()
        s1.release()
        zdrp.release()
        upool.release()
        cpool.release()

    _CACHE["nc"] = nc
    return nc


# ------------------------------------------------------------------ kernel --
def kernel(**inputs):
    x = np.asarray(inputs["x"], np.float32)
    W_in = np.asarray(inputs["W_in"], np.float32)
    conv_w = np.asarray(inputs["conv_w"], np.float32)
    conv_b = np.asarray(inputs["conv_b"], np.float32)
    x_proj_w = np.asarray(inputs["x_proj_w"], np.float32)
    dt_w = np.asarray(inputs["dt_w"], np.float32)
    dt_b = np.asarray(inputs["dt_b"], np.float32)
    A_logs = np.asarray(inputs["A_logs"], np.float32)
    Ds = np.asarray(inputs["Ds"], np.float32)
    ln_w = np.asarray(inputs["ln_w"], np.float32)
    ln_b = np.asarray(inputs["ln_b"], np.float32)
    W_out = np.asarray(inputs["W_out"], np.float32)

    nc = _build()
    from concourse.bass_utils import run_bass_kernel_spmd

    def pack_cols(v):  # [K?, DI] -> [P, DT] column per d-tile
        return np.ascontiguousarray(v.reshape(DT, P).T)

    def _pad_xproj(xpw):
        t = xpw.transpose(0, 2, 1)                              # [K, DI, 48]
        out = np.zeros((K, DI, 96), np.float16)
        out[:, :, 0:16] = t[:, :, 0:16]
        out[:, :, 32:48] = t[:, :, 16:32]
        out[:, :, 64:80] = t[:, :, 32:48]
        return out

    xTb = np.ascontiguousarray(x.transpose(0, 2, 1))           # [B, DIM, L]
    A = (-np.exp(A_logs)).reshape(K, DI, N)
    A_pack = np.stack([
        np.ascontiguousarray(
            A[k].reshape(64, 8, N).transpose(2, 1, 0).reshape(P, 64))
        for k in range(K)])
    l8 = np.zeros((8, P), np.float16)
    for p in range(P):
        l8[p % 8, p] = 1.0
    # variant q (cols [32q,32q+32)): one-hot at local col 8q + p%8
    l32 = np.zeros((P, P), np.float16)
    for q in range(4):
        for p in range(P):
            l32[p, 32 * q + 8 * q + (p % 8)] = 1.0
    Ds2 = Ds.reshape(K, DI)
    ldiag = np.zeros((K, DT, P, P), np.float16)
    for k in range(K):
        for sp in range(DT):
            np.fill_diagonal(ldiag[k, sp], Ds2[k, sp * P : (sp + 1) * P])
    shared = {
        "winT": np.ascontiguousarray(W_in.T).astype(np.float16),
        "convT": np.ascontiguousarray(conv_w.T).astype(np.float16),
        "convb": pack_cols(conv_b),
        "nconvb": pack_cols(-conv_b),
        "xprojT": _pad_xproj(x_proj_w),                     # [K, DI, 96]
        "dtwT": np.ascontiguousarray(
            dt_w.transpose(0, 2, 1)).astype(np.float16),        # [K, R, DI]
        "dtb": np.stack([pack_cols(dt_b[k]) for k in range(K)]),
        "A": A_pack,
        "Ds": np.stack([pack_cols(Ds.reshape(K, DI)[k]) for k in range(K)]),
        "lnw": pack_cols(ln_w),
        "lnb": pack_cols(ln_b),
        "woutT": np.ascontiguousarray(W_out.T).astype(np.float16),  # [DI, DIM]
        "ones": np.ones((P, 1), np.float16),
        "onesr": np.ones((1, P), np.float16),
        "eps": np.full((P, 1), 1e-5, np.float32),
        "l8": l8,
        "l32": l32,
        "ldiag": ldiag,
    }
    in_maps = [dict(shared, xT=np.ascontiguousarray(xTb[b]).astype(np.float16))
               for b in range(B)]

    trace = _install_profile_shim()
    try:
        res = run_bass_kernel_spmd(
            nc, in_maps, core_ids=list(range(B)), trace=trace)
    except Exception:
        if not trace:
            raise
        res = run_bass_kernel_spmd(
            nc, in_maps, core_ids=list(range(B)), trace=False)
    if res.exec_time_ns is not None:
        print(f"HW exec time: {res.exec_time_ns} ns")

    out = np.stack([res.results[b]["out"].T for b in range(B)])  # [B, L, DIM]
    return out.astype(np.float32)


if __name__ == "__main__":
    nc = _build()
    n_inst = sum(
        len(blk.instructions) for fn in nc.m.functions for blk in fn.blocks)
    print("built ok, instructions:", n_inst)

